# revision 40
# baseline (speedup 1.0000x reference)
"""MultiHeadAttention Trainium2 Bass kernel, 8-core SPMD.

Problem: B=4, S=2048, EMBED=1024, HEADS=16, HEAD_DIM=64 (fp32).

Sharding: core c -> batch b=c//2, head-half hh=c%2 (8 local heads,
Megatron-style tensor parallel). Each core computes the FULL 2048-query
attention for its 8 heads plus the partial output projection through its
512 columns of Wo; the host sums the two partials per batch (the
row-parallel all-reduce done at unshard time). No duplicated K/V
projection work and no device collectives.

All matmuls run in bf16 (1 PE cycle/row at any tile size); PSUM
accumulation stays f32. Per-core PE work: Q/K/V proj 3x27.3us +
attention 2x109.2us + O proj 27.3us = 327.6us, vs 398.8us for the
(batch, query-half) baseline -- the K/V duplication is gone.

Structure: a fused software pipeline over head pairs p=0..3. Round p
runs attention for pair p (ACT-heavy: exp) with the K/Q projections for
pair p+1 (PE-only) interleaved into the inner loop, so the PE fills the
gaps where exp is the per-iteration critical path. The V projection for
all 8 heads streams into round 0 the same way; the output projection for
q-chunk qc streams into round 3's chunk qc+1.

Per-core dataflow (feature/contraction dim on partitions):
  kt_p [128=2x64 dh, 2048 kk] = (Wk_p.T @ Xk + bk_p)/8, bf16
  qt_p [128, 2048 q]          = Wq_p.T @ Xq + bq_p, bf16
  vaug [128 kk, kkt, 8h x 65] = V bf16 + ones column per head (PV then
                                yields the softmax denominator free)
  per (qc 512q, kkp 2x128kk): S.T[kk,q] = kt_h.T @ qt_h (PE quadrants),
    P = exp(S.T) (ACT) -> bf16, P *= notm (DVE), OT[65,512] += Vaug.T @ P
  normalize: OT[0:64] * recip(OT[64]) -> ot_sb bf16 (stays in SBUF)
  outT_partial = Wo_l.T @ OT + bo2   (bo2 = Wo_l@bv_l + bo on core 0)
Host: out[b] = (outT[2b] + outT[2b+1]).T
"""
import numpy as np
import ml_dtypes

import concourse.bass as bass
import concourse.mybir as mybir
import concourse.tile as tile
from concourse import bacc
from concourse.bass_utils import run_bass_kernel_spmd

F32 = mybir.dt.float32
BF16 = mybir.dt.bfloat16
Act = mybir.ActivationFunctionType
Alu = mybir.AluOpType

EMBED = 1024
HEADS = 16
LH = 8        # local heads per core
PAIRS = 4     # local head pairs
DH = 64
SQ = 2048     # query rows (full)
SK = 2048     # key rows (full)
NF = 8        # contraction feature tiles (1024/128)
KKT = 16      # kk tiles of 128
QC = 4        # q chunks of 512
KKP = 8       # kk super-tiles of 256 (2 j x 128)
N_CORES = 8

_STATE = {}


def build_nc():
    nc = bacc.Bacc("TRN2", target_bir_lowering=False)
    xqT = nc.dram_tensor("xqT", [EMBED, SQ], BF16, kind="ExternalInput")
    xkT = nc.dram_tensor("xkT", [EMBED, SK], BF16, kind="ExternalInput")
    xvT = nc.dram_tensor("xvT", [EMBED, SK], BF16, kind="ExternalInput")
    wqT = nc.dram_tensor("wqT", [EMBED, 512], BF16, kind="ExternalInput")
    wkT = nc.dram_tensor("wkT", [EMBED, 512], BF16, kind="ExternalInput")
    wvT = nc.dram_tensor("wvT", [EMBED, 512], BF16, kind="ExternalInput")
    woT = nc.dram_tensor("woT", [512, EMBED], BF16, kind="ExternalInput")
    bq_l = nc.dram_tensor("bq_l", [128, PAIRS], F32, kind="ExternalInput")
    bk8_l = nc.dram_tensor("bk8_l", [128, PAIRS], F32, kind="ExternalInput")
    bo2_l = nc.dram_tensor("bo2_l", [128, NF], F32, kind="ExternalInput")
    notmT = nc.dram_tensor("notmT", [SK, SQ], BF16, kind="ExternalInput")
    outT = nc.dram_tensor("outT", [EMBED, SQ], F32, kind="ExternalOutput")

    xqT_r = xqT.rearrange("(t p) q -> p t q", p=128)
    xkT_r = xkT.rearrange("(t p) k -> p t k", p=128)
    xvT_r = xvT.rearrange("(t p) k -> p t k", p=128)
    wqT_r = wqT.rearrange("(t p) n -> p t n", p=128)
    wkT_r = wkT.rearrange("(t p) n -> p t n", p=128)
    wvT_r = wvT.rearrange("(t p) n -> p t n", p=128)
    woT_r = woT.rearrange("(t p) n -> p t n", p=128)
    notmT_r = notmT.rearrange("(t p) q -> p t q", p=128)

    with tile.TileContext(nc) as tc:
        with tc.tile_pool(name="bias", bufs=1) as bp, \
             tc.tile_pool(name="persist", bufs=1) as pp, \
             tc.tile_pool(name="ktqt", bufs=2) as kq, \
             tc.tile_pool(name="xq", bufs=2) as xqp, \
             tc.tile_pool(name="wkq", bufs=2) as wkqp, \
             tc.tile_pool(name="pt", bufs=2) as ptp, \
             tc.tile_pool(name="nrm", bufs=2) as nrmp, \
             tc.tile_pool(name="xk", bufs=1) as xkp, \
             tc.tile_pool(name="bst", bufs=1, space="PSUM") as stpool, \
             tc.tile_pool(name="bot", bufs=1, space="PSUM") as otpool, \
             tc.tile_pool(name="proj", bufs=2, space="PSUM") as projp:
            bq_sb = bp.tile([128, PAIRS], F32, name="bq_sb")
            bk8_sb = bp.tile([128, PAIRS], F32, name="bk8_sb")
            bo2_sb = bp.tile([128, NF], F32, name="bo2_sb")
            notm = pp.tile([128, KKT, SQ], BF16, name="notm")
            vaug = pp.tile([128, KKT, LH * 65], BF16, name="vaug")
            vaug_r = vaug.rearrange("p k (h c) -> p k h c", c=65)
            ot_sb = pp.tile([128, PAIRS, SQ], BF16, name="ot_sb")
            xk_sb = xkp.tile([128, NF, SK], BF16, name="xk_sb")

            kt_t = {}
            qt_t = {}

            def alloc_ktqt(p):
                kt_t[p] = kq.tile([128, SK], BF16, name=f"kt{p}", tag="kt")
                qt_t[p] = kq.tile([128, SQ], BF16, name=f"qt{p}", tag="qt")

            wk_sb = {}
            wq_sb = {}

            def load_wk(p, queue):
                t = wkqp.tile([128, NF, 128], BF16, name=f"wk{p}", tag="wk")
                queue.dma_start(out=t[:],
                                in_=wkT_r[:, :, p * 128:(p + 1) * 128])
                wk_sb[p] = t

            def load_wq(p, queue):
                t = wkqp.tile([128, NF, 128], BF16, name=f"wq{p}", tag="wq")
                queue.dma_start(out=t[:],
                                in_=wqT_r[:, :, p * 128:(p + 1) * 128])
                wq_sb[p] = t

            xq_chunks = {}

            def load_xq(p, qc, queue):
                t = xqp.tile([128, NF, 512], BF16, name="xq_ch", tag="xq")
                queue.dma_start(
                    out=t[:], in_=xqT_r[:, :, qc * 512:(qc + 1) * 512])
                xq_chunks[(p, qc)] = t

            # ---------- instruction-level group builders ----------
            def kproj_group(p, kkc):
                ps = projp.tile([128, 512], F32, name="kps", tag="proj")
                ops = []
                for fi in range(NF):
                    ops.append(lambda fi=fi, ps=ps: nc.tensor.matmul(
                        ps[:], wk_sb[p][:, fi, :],
                        xk_sb[:, fi, kkc * 512:(kkc + 1) * 512],
                        start=(fi == 0), stop=(fi == NF - 1)))
                ops.append(lambda ps=ps: nc.vector.tensor_scalar(
                    out=kt_t[p][:, kkc * 512:(kkc + 1) * 512], in0=ps[:],
                    scalar1=0.125, scalar2=bk8_sb[:, p:p + 1],
                    op0=Alu.mult, op1=Alu.add))
                return ops

            def qproj_group(p, qc):
                ps = projp.tile([128, 512], F32, name="qps", tag="proj")
                xq_ch = xq_chunks[(p, qc)]
                ops = []
                for fi in range(NF):
                    ops.append(lambda fi=fi, ps=ps: nc.tensor.matmul(
                        ps[:], wq_sb[p][:, fi, :], xq_ch[:, fi, :],
                        start=(fi == 0), stop=(fi == NF - 1)))
                ops.append(lambda ps=ps: nc.vector.tensor_scalar(
                    out=qt_t[p][:, qc * 512:(qc + 1) * 512], in0=ps[:],
                    scalar1=bq_sb[:, p:p + 1], scalar2=None, op0=Alu.add))
                return ops

            def vproj_group(wv_sb, kkt, xv_ch, sub, h0=0, nh=LH):
                # xv_ch covers kk tiles [2c, 2c+1]; sub selects which.
                # h0/nh select a contiguous local-head range of V columns.
                ps = projp.tile([128, nh * 64], F32, name="vps", tag="proj")
                ops = []
                for fi in range(NF):
                    ops.append(lambda fi=fi, ps=ps: nc.tensor.matmul(
                        ps[:], xv_ch[:, fi, sub * 128:(sub + 1) * 128],
                        wv_sb[:, fi, h0 * 64:(h0 + nh) * 64],
                        start=(fi == 0), stop=(fi == NF - 1)))
                ops.append(lambda ps=ps: nc.vector.tensor_copy(
                    out=vaug_r[:, kkt, h0:h0 + nh, 0:64],
                    in_=ps.rearrange("p (h c) -> p h c", c=64)))
                return ops

            def oproj_group(wo_sb, m, qc, cstage):
                ps = projp.tile([128, 512], F32, name="ops", tag="proj")
                ops = []
                for fp in range(PAIRS):
                    ops.append(lambda fp=fp, ps=ps: nc.tensor.matmul(
                        ps[:], wo_sb[:, fp, m * 128:(m + 1) * 128],
                        ot_sb[:, fp, qc * 512:(qc + 1) * 512],
                        start=(fp == 0), stop=(fp == PAIRS - 1)))

                def evac(ps=ps):
                    stg = cstage.tile([128, 512], F32, name="cstg")
                    nc.vector.tensor_scalar(
                        out=stg[:], in0=ps[:], scalar1=bo2_sb[:, m:m + 1],
                        scalar2=None, op0=Alu.add)
                    nc.sync.dma_start(
                        out=outT[m * 128:(m + 1) * 128,
                                 qc * 512:(qc + 1) * 512],
                        in_=stg[:])
                ops.append(evac)
                return ops

            # ---------- attention inner iteration ----------
            def attn_iter(p, qc, kkp, otps, extra):
                for fn in extra:
                    fn()
                sts = [stpool.tile([128, 1024], F32, name=f"stps{j}",
                                   tag=f"stps{j}") for j in range(2)]

                def st_mm(hh, j):
                    lo = hh * 64
                    kkt = 2 * kkp + j
                    nc.tensor.matmul(
                        sts[hh][:, j * 512:(j + 1) * 512],
                        kt_t[p][lo:lo + 64, kkt * 128:(kkt + 1) * 128],
                        qt_t[p][lo:lo + 64, qc * 512:(qc + 1) * 512],
                        start=True, stop=True, tile_position=(lo, 0))

                pts = []
                st_mm(0, 0)
                st_mm(1, 0)
                st_mm(0, 1)
                pt0 = ptp.tile([128, 1024], BF16, name="pt0", tag="pt0")
                nc.scalar.activation(pt0[:], sts[0][:], Act.Exp)
                pts.append(pt0)
                st_mm(1, 1)
                pt1 = ptp.tile([128, 1024], BF16, name="pt1", tag="pt1")
                nc.scalar.activation(pt1[:], sts[1][:], Act.Exp)
                pts.append(pt1)
                for hh in range(2):
                    nc.vector.tensor_tensor(
                        out=pts[hh].rearrange("p (j q) -> p j q", q=512),
                        in0=pts[hh].rearrange("p (j q) -> p j q", q=512),
                        in1=notm[:, 2 * kkp:2 * kkp + 2,
                                 qc * 512:(qc + 1) * 512],
                        op=Alu.mult)
                for hh in range(2):
                    for j in range(2):
                        kkt = 2 * kkp + j
                        nc.tensor.matmul(
                            otps[hh][0:65, :],
                            vaug_r[:, kkt, 2 * p + hh, :],
                            pts[hh][:, j * 512:(j + 1) * 512],
                            start=(kkp == 0 and j == 0),
                            stop=(kkp == KKP - 1 and j == 1))

            def normalize(p, qc, otps):
                for hh in range(2):
                    rec = nrmp.tile([1, 512], F32, name="rec", tag="rec")
                    nc.vector.reciprocal(rec[:], otps[hh][64:65, :])
                    recb = nrmp.tile([64, 512], F32, name="recb", tag="recb")
                    nc.gpsimd.partition_broadcast(recb[:], rec[:])
                    nc.vector.tensor_tensor(
                        out=ot_sb[hh * 64:(hh + 1) * 64, p,
                                  qc * 512:(qc + 1) * 512],
                        in0=otps[hh][0:64, :], in1=recb[:], op=Alu.mult)

            def run_round(p, slots):
                for qc in range(QC):
                    otps = [otpool.tile([128, 512], F32, name=f"otps{h}",
                                        tag=f"otps{h}") for h in range(2)]
                    for kkp in range(KKP):
                        attn_iter(p, qc, kkp, otps,
                                  slots.get((qc, kkp), []))
                    normalize(p, qc, otps)

            # ---------- prologue ----------
            nc.sync.dma_start(out=bq_sb[:], in_=bq_l[:, :])
            nc.sync.dma_start(out=bk8_sb[:], in_=bk8_l[:, :])
            nc.sync.dma_start(out=bo2_sb[:], in_=bo2_l[:, :])
            nc.vector.memset(vaug_r[:, :, :, 64:65], 1.0)

            with tc.tile_pool(name="xv", bufs=2) as xvp, \
                 tc.tile_pool(name="wv", bufs=1) as wvp:
                load_wk(0, nc.sync)
                for kkc in range(4):
                    nc.sync.dma_start(
                        out=xk_sb[:, :, kkc * 512:(kkc + 1) * 512],
                        in_=xkT_r[:, :, kkc * 512:(kkc + 1) * 512])
                load_wq(0, nc.sync)

                wv_sb = wvp.tile([128, NF, 512], BF16, name="wv_sb")
                nc.gpsimd.dma_start(out=wv_sb[:], in_=wvT_r[:, :, :])
                xv_chunks = {}

                def load_xv(c, queue):
                    t = xvp.tile([128, NF, 256], BF16, name="xv_ch", tag="xv")
                    queue.dma_start(
                        out=t[:], in_=xvT_r[:, :, c * 256:(c + 1) * 256])
                    xv_chunks[c] = t

                load_xv(0, nc.gpsimd)
                nc.gpsimd.dma_start(out=notm[:, 0:4, :],
                                    in_=notmT_r[:, 0:4, :])
                nc.gpsimd.dma_start(out=notm[:, 4:8, :],
                                    in_=notmT_r[:, 4:8, :])
                for qc in range(QC):
                    load_xq(0, qc, nc.gpsimd)
                load_xv(1, nc.gpsimd)
                nc.gpsimd.dma_start(out=notm[:, 8:12, :],
                                    in_=notmT_r[:, 8:12, :])
                nc.gpsimd.dma_start(out=notm[:, 12:16, :],
                                    in_=notmT_r[:, 12:16, :])

                # prologue compute: K/Q proj for pair 0, V proj kkt 0,1
                alloc_ktqt(0)
                for kkc in range(4):
                    for fn in kproj_group(0, kkc):
                        fn()
                for qc in range(QC):
                    for fn in qproj_group(0, qc):
                        fn()
                for kkt in range(2):
                    for fn in vproj_group(wv_sb, kkt, xv_chunks[0], kkt):
                        fn()

                # ---------- round 0 ----------
                slots = {}
                for i in range(7):
                    g = []
                    for kkt in (2 * i + 2, 2 * i + 3):
                        c = kkt // 2
                        if c not in xv_chunks:
                            load_xv(c, nc.gpsimd)
                        g += vproj_group(wv_sb, kkt, xv_chunks[c], kkt % 2)
                    slots[(0, i)] = g
                alloc_ktqt(1)
                load_wk(1, nc.sync)
                load_wq(1, nc.sync)
                for qc in range(QC):
                    load_xq(1, qc, nc.gpsimd)
                groups = [kproj_group(1, kkc) for kkc in range(4)]
                groups += [qproj_group(1, qc) for qc in range(QC)]
                for gi, g in enumerate(groups):
                    slots.setdefault(divmod(8 + 3 * gi, KKP), []).extend(g)
                run_round(0, slots)

                # ---------- rounds 1..2 ----------
                for p in (1, 2):
                    nxt = p + 1
                    alloc_ktqt(nxt)
                    load_wk(nxt, nc.sync)
                    load_wq(nxt, nc.sync)
                    slots = {}
                    for qc in range(QC):
                        load_xq(nxt, qc, nc.gpsimd)
                    groups = [kproj_group(nxt, kkc) for kkc in range(4)]
                    groups += [qproj_group(nxt, qc) for qc in range(QC)]
                    if p == 2:
                        # defer kproj(3,3)/qproj(3,3) into round 3's
                        # otherwise-empty first chunk
                        groups = groups[:3] + groups[4:7]
                    for gi, g in enumerate(groups):
                        slots.setdefault(divmod(8 + 3 * gi, KKP), []).extend(g)
                    run_round(p, slots)

            # ---------- round 3 ----------
            with tc.tile_pool(name="wo", bufs=1) as wop, \
                 tc.tile_pool(name="cs", bufs=3) as cstage:
                wo_sb = wop.tile([128, PAIRS, EMBED], BF16, name="wo_sb")
                nc.gpsimd.dma_start(out=wo_sb[:], in_=woT_r[:, :, :])

                # qc3's output projection is split: pairs 0-2 (whose qc3
                # attention outputs landed in rounds 0-2) accumulate during
                # round-3 chunk 0 into bf16 partials; only the pair-3 matmul
                # + add remains after the final normalize, shrinking the
                # drain tail.
                with tc.tile_pool(name="osp", bufs=1) as osp:
                    part_sb = [osp.tile([128, 512], BF16, name=f"part{m}",
                                        tag=f"part{m}") for m in range(NF)]

                    def oproj_a(m):
                        ps = projp.tile([128, 512], F32, name="oa",
                                        tag="proj")
                        ops = []
                        for fp in range(3):
                            ops.append(lambda fp=fp, ps=ps: nc.tensor.matmul(
                                ps[:], wo_sb[:, fp, m * 128:(m + 1) * 128],
                                ot_sb[:, fp, 3 * 512:4 * 512],
                                start=(fp == 0), stop=(fp == 2)))
                        ops.append(lambda ps=ps: nc.vector.tensor_scalar(
                            out=part_sb[m][:], in0=ps[:],
                            scalar1=bo2_sb[:, m:m + 1], scalar2=None,
                            op0=Alu.add))
                        return ops

                    def oproj_b(m):
                        ps = projp.tile([128, 512], F32, name="ob",
                                        tag="proj")
                        nc.tensor.matmul(
                            ps[:], wo_sb[:, 3, m * 128:(m + 1) * 128],
                            ot_sb[:, 3, 3 * 512:4 * 512],
                            start=True, stop=True)
                        stg = cstage.tile([128, 512], F32, name="cstg")
                        nc.vector.tensor_tensor(
                            out=stg[:], in0=ps[:], in1=part_sb[m][:],
                            op=Alu.add)
                        nc.sync.dma_start(
                            out=outT[m * 128:(m + 1) * 128, 3 * 512:4 * 512],
                            in_=stg[:])

                    # round 3: interleave the output projection
                    slots = {(0, 0): kproj_group(3, 3),
                             (0, 2): qproj_group(3, 3)}
                    am = iter(range(NF))
                    for kkp in (1, 3, 4, 5, 6, 7):
                        slots[(0, kkp)] = oproj_a(next(am))
                    for kkp in (0, 1):
                        slots.setdefault((1, kkp), []).extend(
                            oproj_a(next(am)))
                    for qcd in range(3):  # oproj(qc) during chunk qc+1
                        for m in range(NF):
                            slots.setdefault((qcd + 1, m), []).extend(
                                oproj_group(wo_sb, m, qcd, cstage))
                    run_round(3, slots)
                    for m in range(NF):
                        oproj_b(m)
    nc.compile()
    return nc


def _get_nc():
    if "nc" not in _STATE:
        _STATE["nc"] = build_nc()
    return _STATE["nc"]


BF = ml_dtypes.bfloat16


def kernel(query, key, value, mask, Wq, bq, Wk, bk, Wv, bv, Wo, bo):
    query = np.asarray(query, dtype=np.float32)
    key = np.asarray(key, dtype=np.float32)
    value = np.asarray(value, dtype=np.float32)
    mask = np.asarray(mask)
    Wq = np.asarray(Wq, dtype=np.float32)
    Wk = np.asarray(Wk, dtype=np.float32)
    Wv = np.asarray(Wv, dtype=np.float32)
    Wo = np.asarray(Wo, dtype=np.float32)
    bq = np.asarray(bq, dtype=np.float32)
    bk = np.asarray(bk, dtype=np.float32)
    bv = np.asarray(bv, dtype=np.float32)
    bo = np.asarray(bo, dtype=np.float32)

    wqT = Wq.T  # [in 1024, out 1024]
    wkT = Wk.T
    wvT = Wv.T
    woT = Wo.T  # [in 1024 (concat heads), out 1024]

    xT = {}
    nmT = {}
    for b in range(4):
        xT[("q", b)] = np.ascontiguousarray(query[b].T.astype(BF))
        xT[("k", b)] = np.ascontiguousarray(key[b].T.astype(BF))
        xT[("v", b)] = np.ascontiguousarray(value[b].T.astype(BF))
        nmT[b] = np.ascontiguousarray((~mask[b, 0]).T.astype(BF))

    half = {}
    for hh in range(2):
        cols = slice(hh * 512, (hh + 1) * 512)
        bo2 = woT[cols, :].T @ bv[cols]
        if hh == 0:
            bo2 = bo2 + bo
        half[hh] = {
            "wqT": np.ascontiguousarray(wqT[:, cols].astype(BF)),
            "wkT": np.ascontiguousarray(wkT[:, cols].astype(BF)),
            "wvT": np.ascontiguousarray(wvT[:, cols].astype(BF)),
            "woT": np.ascontiguousarray(woT[cols, :].astype(BF)),
            "bq_l": np.ascontiguousarray(
                bq[cols].reshape(PAIRS, 128).T.astype(np.float32)),
            "bk8_l": np.ascontiguousarray(
                (bk[cols] / 8.0).reshape(PAIRS, 128).T.astype(np.float32)),
            "bo2_l": np.ascontiguousarray(
                bo2.reshape(NF, 128).T.astype(np.float32)),
        }

    in_maps = []
    for c in range(N_CORES):
        b, hh = c // 2, c % 2
        m = {"xqT": xT[("q", b)], "xkT": xT[("k", b)], "xvT": xT[("v", b)],
             "notmT": nmT[b]}
        m.update(half[hh])
        in_maps.append(m)

    nc = _get_nc()
    res = run_bass_kernel_spmd(nc, in_maps, core_ids=list(range(N_CORES)))
    out = np.empty((4, 2048, EMBED), dtype=np.float32)
    for b in range(4):
        acc = res.results[2 * b]["outT"] + res.results[2 * b + 1]["outT"]
        out[b] = acc.T
    return out
